# Initial kernel scaffold
#
"""Chamfer distance kernel for Trainium2 (8 NeuronCores, SPMD).

Reference computation:
    p1 = pc1.reshape(-1, 3)  [N1=16384, 3]
    p2 = pc2.reshape(-1, 3)  [N2=16384, 3]
    d[i, j] = ||p1_i - p2_j||
    out = mean_j(min_i d[i,j]) + mean_i(min_j d[i,j])

Strategy:
  - Shard pc2 rows across 8 cores (2048 points each). Each core computes
    its full distance tile against all of pc1, in both orientations:
      A: [pc1-block=128 part, pc2-shard=2048 free] -> free-min = partial
         col-min (dist2 path), all-min'd across cores on the host.
      B: [pc2-block=128 part, pc1=16384 free]      -> free-min = complete
         row-min (dist1 path) for this core's shard.
  - sqrt is monotone, so mins are taken on squared distances; sqrt and the
    two means run on the host over 8*(16384+2048) partial mins (tiny).
  - SCALE*d2[i,j] = SCALE*(sq1[i] + sq2[j] - 2*dot(p1_i, p2_j)) is
    produced directly by one K=24 augmented matmul per tile: 18 rows give
    the double-compensated bf16 dot product (hi/mid/lo splits; error
    ~2.5e-7 instead of bf16's 2^-8 -- needed because the true NN d2 here
    is ~5e-5), 6 rows add sq1/sq2 (each split hi+mid+lo). PSUM then holds
    full fp32 SCALE*d2.
  - Each [128, 2048] PSUM tile becomes a [128, 1] row-min via one of two
    routes, interleaved 1:13 so PE (~474us), DVE (~469us) and ScalarE
    (~470us) all finish together at the PE's measured 1.2GHz floor:
    DVE tensor_reduce straight from PSUM (1x, ~2.24us), or ScalarE
    PSUM->SBUF fp16 copy (~1.85us; the x512 pre-scale keeps d2 mins in
    fp16 normal range) + DVE in-place pairwise-min folds at 2x + short
    reduce (~1.5us).
  - Matmul operands must sit at a 32-partition base (0/32/64), so the 8
    pc1 column-groups of 24 contraction rows are packed at bases
    {0,32,64} x 3 column-regions of [128, 6144] SBUF tensors; the small
    pc2-side operands are replicated at all 3 bases.
  - Walrus accepts only one sem-wait per compute instruction; Tile emits
    more on recycled tile slots. _legalize_waits strips transitively
    implied same-engine waits and splits the rest onto injected NoOps.
"""

import os
import sys

import numpy as np

for _p in ("/opt/trn_rl_repo",):
    if os.path.isdir(_p) and _p not in sys.path:
        sys.path.append(_p)

import ml_dtypes

import concourse.bass as bass
import concourse.mybir as mybir
import concourse.tile as tile
from concourse.bass_utils import run_bass_kernel_spmd

BF16 = ml_dtypes.bfloat16

N_CORES = 8
N1 = 16384            # total pc1 points
N_SHARD = 2048        # pc2 points per core
N_GROUPS = 8          # pc1 column-groups
GROUP_COLS = N1 // N_GROUPS  # 2048
K = 24                # augmented contraction depth
MM_N = 512            # matmul moving free dim (one PSUM bank of fp32)
SCALE = 512.0         # power-of-two scale on d2 (fp16 normal range)
DIRECT_EVERY = 13     # 1-in-N tiles reduced straight from PSUM (0 = none)
IN_COLS = 6 * GROUP_COLS + 2 * N_SHARD  # packed input columns (16384)

TRACE = False         # test harness can flip this for profiled runs
LAST_RESULTS = None   # stashed BassKernelResults for the test harness

_NC_CACHE = None


def _build_nc():
    """Build the per-core Bass module (same NEFF on all 8 cores)."""
    nc = bass.Bass(trn_type="TRN2")

    # Packed input, cols: [0:6144) p1w, [6144:12288) p1m,
    # [12288:14336) p2w, [14336:16384) p2m.
    inp = nc.dram_tensor("inp", [128, IN_COLS], mybir.dt.bfloat16,
                         kind="ExternalInput")
    # Packed output (SCALE*d2 partial mins):
    # mout[:, 0:128]   = m2[p, bi]: min over this core's pc2 shard for
    #                    pc1 point bi*128+p (host mins across cores).
    # mout[:, 128:256] = raw per-(bj, g) row-mins for pc2_shard point
    #                    bj*128+p vs pc1 group g (host mins over g).
    mout = nc.dram_tensor("mout", [128, N1 // 128 + N_SHARD // 128 * N_GROUPS],
                          mybir.dt.float32, kind="ExternalOutput")

    with tile.TileContext(nc) as tc:
        with (
            tc.tile_pool(name="ins", bufs=1) as ins_pool,
            tc.tile_pool(name="psum", bufs=2, space="PSUM") as psum_pool,
            tc.tile_pool(name="outs", bufs=1) as out_pool,
            tc.tile_pool(name="f16", bufs=8) as f16_pool,
        ):
            inp_sb = ins_pool.tile([128, IN_COLS], mybir.dt.bfloat16,
                                   tag="inp")
            # Four dma_starts land on different HWDGE queues and run
            # concurrently -> input load completes in about a quarter the
            # time (nothing else is running yet, so no port contention).
            q = IN_COLS // 4
            for qi in range(4):
                nc.sync.dma_start(inp_sb[:, qi * q:(qi + 1) * q],
                                  inp[:, qi * q:(qi + 1) * q])
            p1w_sb = inp_sb[:, 0:3 * GROUP_COLS]
            p1m_sb = inp_sb[:, 3 * GROUP_COLS:6 * GROUP_COLS]
            p2w_sb = inp_sb[:, 6 * GROUP_COLS:6 * GROUP_COLS + N_SHARD]
            p2m_sb = inp_sb[:, 6 * GROUP_COLS + N_SHARD:IN_COLS]

            mout_sb = out_pool.tile(
                [128, N1 // 128 + N_SHARD // 128 * N_GROUPS],
                mybir.dt.float32, tag="mout")
            m2_sb = mout_sb[:, 0:N1 // 128]
            # Raw per-(bj, g) row-mins; the min over g happens on the host
            # (saves 16 DVE second-level reduces + their per-bj dep chains).
            m1_sb = mout_sb[:, N1 // 128:]

            def grp(sb, g, c0, c1):
                """K-row slice of a group-packed pc1-side tensor."""
                q, h = g % 3, g // 3
                return sb[32 * q:32 * q + K, h * GROUP_COLS + c0:h * GROUP_COLS + c1]

            def rep(sb, g, c0, c1):
                """K-row slice of a base-replicated pc2-side tensor."""
                q = g % 3
                return sb[32 * q:32 * q + K, c0:c1]

            # Two ways to turn a PSUM tile into a [128,1] row-min:
            #  - DVE-direct: tensor_reduce(min) from PSUM fp32 (1x rate,
            #    ~2.24us/tile).
            #  - ACT-route: ScalarE copies PSUM -> SBUF fp16 (~1.85us; the
            #    x512 pre-scale keeps d2 mins in fp16 normal range), then
            #    DVE runs three in-place pairwise-min folds at 2x_1P rate
            #    plus a short 1x reduce (~1.44us total) -- 0.8us cheaper
            #    per tile than direct.
            # Route 1-in-DIRECT_EVERY direct so DVE and ACT drain at
            # matched rates.
            tile_idx = [0]

            def consume(pt, target):
                i = tile_idx[0]
                tile_idx[0] += 1
                if DIRECT_EVERY and i % DIRECT_EVERY < 1:
                    nc.vector.tensor_reduce(
                        out=target, in_=pt[:],
                        axis=mybir.AxisListType.X, op=mybir.AluOpType.min,
                    )
                else:
                    f16 = f16_pool.tile([128, N_SHARD], mybir.dt.float16,
                                        tag="f16")
                    nc.scalar.copy(f16[:], pt[:])
                    for half in (1024, 512, 256):
                        nc.vector.tensor_tensor(
                            out=f16[:, :half], in0=f16[:, :half],
                            in1=f16[:, half:2 * half],
                            op=mybir.AluOpType.min,
                        )
                    nc.vector.tensor_reduce(
                        out=target, in_=f16[:, :256],
                        axis=mybir.AxisListType.X, op=mybir.AluOpType.min,
                    )

            # Orientation A: 128 pc1-blocks; free dim = pc2 shard (2048).
            for bi in range(N1 // 128):
                g, b_in = divmod(bi, GROUP_COLS // 128)
                lhsT = grp(p1w_sb, g, b_in * 128, (b_in + 1) * 128)
                pt = psum_pool.tile([128, N_SHARD], mybir.dt.float32, tag="ps")
                for c in range(N_SHARD // MM_N):
                    nc.tensor.matmul(
                        pt[:, c * MM_N:(c + 1) * MM_N],
                        lhsT,
                        rep(p2m_sb, g, c * MM_N, (c + 1) * MM_N),
                        start=True, stop=True,
                    )
                consume(pt, m2_sb[:, bi:bi + 1])

            # m2 is complete after the A loop: ship it while B computes.
            nc.sync.dma_start(mout[:, 0:N1 // 128], m2_sb[:])

            # Orientation B: 16 pc2-blocks; free dim = all pc1 (8 groups x 2048).
            for bj in range(N_SHARD // 128):
                for g in range(N_GROUPS):
                    lhsT = rep(p2w_sb, g, bj * 128, (bj + 1) * 128)
                    pt = psum_pool.tile([128, GROUP_COLS], mybir.dt.float32,
                                        tag="ps")
                    for c in range(GROUP_COLS // MM_N):
                        nc.tensor.matmul(
                            pt[:, c * MM_N:(c + 1) * MM_N],
                            lhsT,
                            grp(p1m_sb, g, c * MM_N, (c + 1) * MM_N),
                            start=True, stop=True,
                        )
                    consume(pt, m1_sb[:, bj * N_GROUPS + g:bj * N_GROUPS + g + 1])

            nc.sync.dma_start(mout[:, N1 // 128:], m1_sb[:])

    _legalize_waits(nc)
    return nc


def _legalize_waits(nc):
    """Walrus's per-instruction structs carry at most one sem-wait, but
    Tile's sem assignment can emit several (slot-recycle WAR + input RAW).

    1. Same-engine waits are dropped when a cross-engine wait remains:
       engines execute in order and the cross-engine consumer they wait
       on transitively waited on those same-engine ticks.
    2. The kernel-tail Drain waits on every DMA queue + PE + DVE; all of
       it is transitively covered by the single output DMA.
    3. Any instruction still carrying N>1 waits gets N-1 same-engine
       NoOps injected right before it, one overflow wait each.
    """
    import concourse.mybir as mybir

    blocks = nc.m.functions[0].blocks

    # 1. same-engine strip
    for blk in blocks:
        for ins in blk.instructions:
            si = ins.sync_info
            if si is None or len(si.on_wait) <= 1 or not si.on_update:
                continue
            self_eng = si.on_update[0].ant_name.split("_")[0]
            keep = [w for w in si.on_wait
                    if w.ant_name.split("_")[0] != self_eng]
            if keep and len(keep) < len(si.on_wait):
                si.on_wait = keep
                ins.sync_info = si

    # 2. tail drain: keep only the output DMA queue's wait
    out_sems = set()
    for blk in blocks:
        for ins in blk.instructions:
            if type(ins).__name__ == "InstDMACopy" and ins.outs and \
                    getattr(ins.outs[0], "memref", "") == "mout":
                si = ins.sync_info
                for u in (si.on_update if si else []):
                    out_sems.add(u.ant_name)
    for blk in blocks:
        for ins in blk.instructions:
            if type(ins).__name__ != "InstDrain" or not out_sems:
                continue
            si = ins.sync_info
            if si is None or len(si.on_wait) <= 1:
                continue
            keep = [w for w in si.on_wait if w.ant_name in out_sems]
            if keep and len(keep) < len(si.on_wait):
                si.on_wait = keep
                ins.sync_info = si

    # 3. split remaining multi-waits onto same-engine NoOps
    eng_by_prefix = {
        "PE": mybir.EngineType.PE,
        "DVE": mybir.EngineType.DVE,
        "ACT": mybir.EngineType.Activation,
        "POOL": mybir.EngineType.Pool,
        "SP": mybir.EngineType.SP,
    }
    nop_id = [0]
    for blk in blocks:
        new_list = []
        changed = False
        for ins in blk.instructions:
            si = ins.sync_info
            if si is not None and len(si.on_wait) > 1:
                eng = getattr(ins, "engine", None)
                if eng is None and si.on_update:
                    eng = eng_by_prefix.get(
                        si.on_update[0].ant_name.split("_")[0])
                assert eng is not None, \
                    f"{ins.name}: cannot infer engine for wait split"
                waits = list(si.on_wait)
                for w in waits[:-1]:
                    nop_id[0] += 1
                    nop = mybir.InstNoOp(
                        name=f"I-waitnop-{nop_id[0]}", ins=[], outs=[],
                        engine=eng,
                        sync_info=mybir.SyncInfo(on_wait=[w], on_update=[]),
                    )
                    new_list.append(nop)
                si.on_wait = [waits[-1]]
                ins.sync_info = si
                changed = True
            new_list.append(ins)
        if changed:
            blk.instructions = new_list


def _split3(x):
    """fp32 -> three bf16 terms with x ~= h + m + l (residual ~2^-24 |x|)."""
    h = x.astype(BF16)
    r = x - h.astype(np.float32)
    m = r.astype(BF16)
    l = (r - m.astype(np.float32)).astype(BF16)
    return h, m, l


def _prep_side(p):
    """p: [N, 3] fp32 -> (weight_rows [24, N], moving_rows [24, N]).

    Row r of the weight side pairs with row r of the other cloud's moving
    side; the contraction sums, per coordinate, the six hi/mid/lo product
    terms of magnitude >= ~2^-17 (double-compensated bf16 dot, error
    ~2.5e-7), plus three hi/mid/lo rows for each side's |p|^2. The weight
    side carries SCALE (a power of two), so PSUM holds SCALE*d2 exactly
    scaled -- keeping d2 row-mins (~2.5e-5 here) inside fp16 normal range
    for the ACT-routed fp16 evacuation path.
    """
    x, y, z = p[:, 0], p[:, 1], p[:, 2]
    sq = (x * x + y * y + z * z).astype(np.float32)
    w_rows, m_rows = [], []
    for c in (x, y, z):
        h, m, l = _split3(c)
        # (W, M) pairs: (h,h) (m,h) (h,m) (l,h) (m,m) (h,l)
        w_rows += [-2 * SCALE * h, -2 * SCALE * m, -2 * SCALE * h,
                   -2 * SCALE * l, -2 * SCALE * m, -2 * SCALE * h]
        m_rows += [h, h, m, h, m, l]
    ones = np.ones_like(sq)
    w_rows += [SCALE * ones] * 3 + list(_split3(SCALE * sq))
    m_rows += list(_split3(sq)) + [ones] * 3
    return (np.stack(w_rows).astype(BF16), np.stack(m_rows).astype(BF16))


def _group_pack(rows13):
    """[13, N1] -> [128, 6144]: group g at partition base 32*(g%3),
    column region g//3 (AP base partition must be in {0,32,64})."""
    out = np.zeros((128, 3 * GROUP_COLS), dtype=BF16)
    for g in range(N_GROUPS):
        q, h = g % 3, g // 3
        out[32 * q:32 * q + K, h * GROUP_COLS:(h + 1) * GROUP_COLS] = \
            rows13[:, g * GROUP_COLS:(g + 1) * GROUP_COLS]
    return out


def _rep_pack(rows13):
    """[13, N_SHARD] -> [128, N_SHARD]: replicated at bases 0/32/64."""
    out = np.zeros((128, N_SHARD), dtype=BF16)
    for q in range(3):
        out[32 * q:32 * q + K, :] = rows13
    return out


def kernel(pc1, pc2):
    global _NC_CACHE, LAST_RESULTS
    p1 = np.asarray(pc1, dtype=np.float32).reshape(-1, 3)
    p2 = np.asarray(pc2, dtype=np.float32).reshape(-1, 3)
    assert p1.shape == (N1, 3) and p2.shape == (N_CORES * N_SHARD, 3)

    w1, m1rows = _prep_side(p1)
    p1w_np = _group_pack(w1)
    p1m_np = _group_pack(m1rows)

    in_maps = []
    for c in range(N_CORES):
        shard = p2[c * N_SHARD:(c + 1) * N_SHARD]
        w2, m2rows = _prep_side(shard)
        packed = np.concatenate(
            [p1w_np, p1m_np, _rep_pack(w2), _rep_pack(m2rows)], axis=1)
        in_maps.append({"inp": np.ascontiguousarray(packed)})

    if _NC_CACHE is None:
        _NC_CACHE = _build_nc()

    res = run_bass_kernel_spmd(
        _NC_CACHE, in_maps, core_ids=list(range(N_CORES)), trace=TRACE,
    )
    LAST_RESULTS = res

    # m1 per core: complete row-mins of d2 for its 2048 pc2 points.
    # m2 per core: partial col-mins of d2 over its shard -> min across cores.
    nb2 = N1 // 128
    d2_1 = np.concatenate(
        [r["mout"][:, nb2:].reshape(128, N_SHARD // 128, N_GROUPS)
         .min(axis=2).T.reshape(-1) for r in res.results])        # [16384] pc2-major
    d2_2 = np.min(
        np.stack([r["mout"][:, :nb2].T.reshape(-1) for r in res.results]),
        axis=0)                                                   # [16384]

    dist1 = np.sqrt(np.maximum(d2_1 / SCALE, 0.0))
    dist2 = np.sqrt(np.maximum(d2_2 / SCALE, 0.0))
    return np.asarray(dist1.mean() + dist2.mean(), dtype=np.float32)



# revision 5
# speedup vs baseline: 1.0610x; 1.0610x over previous
"""Chamfer distance kernel for Trainium2 (8 NeuronCores, SPMD) — v2.

Reference computation:
    p1 = pc1.reshape(-1, 3)  [N1=16384, 3]
    p2 = pc2.reshape(-1, 3)  [N2=16384, 3]
    d[i, j] = ||p1_i - p2_j||
    out = mean_j(min_i d[i,j]) + mean_i(min_j d[i,j])

v2 strategy (vs the two-orientation baseline at 539us):
  - Compute the 16384x2048 squared-distance block ONCE per core
    (pc2-shard rows on partitions, pc1 on the free axis) and derive BOTH
    reductions from the same PSUM tiles, halving PE work and letting the
    fp32->fp16 conversion be shared:
      * ACT copies each [128,2048] PSUM tile to SBUF fp16 (the only fp32
        scan; 1.85us/tile).
      * DVE row path: racc_bj accumulates elementwise min over the 8 pc1
        groups (fp16 tensor_tensor at 2x); the final group uses
        tensor_tensor_reduce so the full row-min pops out of the same op.
      * DVE/GPSIMD col path: acc_g accumulates elementwise min over the
        16 pc2 blocks; the partition-axis min of acc_g (over 128 rows x 8
        cores) is done on the HOST from the DMA'd [128,16384] fp16 accs
        (device partition reductions are what made the baseline pay two
        full matrix passes).
  - SCALE*d2 produced by one K=24 augmented matmul per 512 cols
    (double-compensated bf16 dot, error ~2.5e-7), as in the baseline.
  - PE warm-up: the HAM clock gate only un-throttles (1.2 -> 2.4 GHz)
    after ~3.4us of CONTINUOUS matmul activity, and the baseline's
    consumer-paced bursts (~1.7us) never tripped it -- that is why its PE
    ran at ~1.1GHz the whole kernel. Here a burst of dummy matmuls during
    the input DMA warms the array; steady-state PE gaps stay well under
    the ~3.4us re-throttle window.
  - Inputs shrink to [24, 18432] bf16 per core (only the contraction rows
    are shipped): ~0.9MB, ~4.5x faster input DMA than the baseline.
  - Walrus accepts only one sem-wait per compute instruction; Tile emits
    more on recycled tile slots. _legalize_waits strips transitively
    implied same-engine waits and splits the rest onto injected NoOps.
"""

import os
import sys

import numpy as np

for _p in ("/opt/trn_rl_repo",):
    if os.path.isdir(_p) and _p not in sys.path:
        sys.path.append(_p)

import ml_dtypes

import concourse.bass as bass
import concourse.mybir as mybir
import concourse.tile as tile
from concourse.bass_utils import run_bass_kernel_spmd

BF16 = ml_dtypes.bfloat16

N_CORES = 8
N1 = 16384            # total pc1 points (free axis)
N_SHARD = 2048        # pc2 points per core (partition axis, 16 blocks)
N_BJ = N_SHARD // 128  # 16 pc2 blocks
N_GROUPS = 8          # pc1 groups
GROUP_COLS = N1 // N_GROUPS  # 2048
K = 24                # augmented contraction depth
MM_N = 512            # matmul moving free dim (one PSUM bank of fp32)
SCALE = 256.0         # power-of-two scale keeps fp16 d2 mins in normal range
BIG = 60000.0         # > SCALE*max(d2), < fp16 max

IN_COLS = N_SHARD + N1  # packed input columns: [0:2048) p2w, [2048:18432) p1m

N_WARM = 26           # dummy matmuls to trip the HAM clock gate warm
GP_EVERY = 0          # 1-in-N col-accs routed to GPSIMD (0 = none)

TRACE = False         # test harness can flip this for profiled runs
LAST_RESULTS = None   # stashed BassKernelResults for the test harness

_NC_CACHE = None


def _build_nc():
    """Build the per-core Bass module (same NEFF on all 8 cores)."""
    nc = bass.Bass(trn_type="TRN2")

    inp = nc.dram_tensor("inp", [K, IN_COLS], mybir.dt.bfloat16,
                         kind="ExternalInput")
    # accs[p, g*2048+f] = min over this core's 16 pc2-blocks of
    # SCALE*d2(pc2 = bj*128+p, pc1 = g*2048+f); host mins over (core, p).
    accs = nc.dram_tensor("accs", [128, N1], mybir.dt.float16,
                          kind="ExternalOutput")
    # m1[p, bj] = full row-min over all pc1 for pc2 point bj*128+p.
    m1 = nc.dram_tensor("m1", [128, N_BJ], mybir.dt.float32,
                        kind="ExternalOutput")

    with tile.TileContext(nc) as tc:
        with (
            tc.tile_pool(name="ins", bufs=1) as ins_pool,
            tc.tile_pool(name="psum", bufs=2, space="PSUM") as psum_pool,
            tc.tile_pool(name="f16", bufs=6) as f16_pool,
            tc.tile_pool(name="racc", bufs=1) as racc_pool,
            tc.tile_pool(name="acc", bufs=2) as acc_pool,
            tc.tile_pool(name="outs", bufs=1) as out_pool,
            tc.tile_pool(name="warm", bufs=1) as warm_pool,
        ):
            inp_sb = ins_pool.tile([K, IN_COLS], mybir.dt.bfloat16, tag="inp")
            # Four parallel HWDGE queues; chunk 0 carries p2w + pc1 group 0
            # so the first tile's operands land first.
            q = IN_COLS // 4
            for qi in range(4):
                nc.sync.dma_start(inp_sb[:, qi * q:(qi + 1) * q],
                                  inp[:, qi * q:(qi + 1) * q])
            p2w_sb = inp_sb[:, 0:N_SHARD]
            p1m_sb = inp_sb[:, N_SHARD:IN_COLS]

            # --- PE warm-up: >=3.4us of back-to-back dummy matmuls while
            # the input DMA streams, so the main loop runs at 2.4 GHz.
            wsrc = warm_pool.tile([32, 128 + MM_N], mybir.dt.bfloat16,
                                  tag="wsrc")
            nc.gpsimd.memset(wsrc[:], 0.0)
            wpt = psum_pool.tile([128, N_SHARD], mybir.dt.float32, tag="ps")
            for _ in range(N_WARM):
                nc.tensor.matmul(
                    wpt[:, 0:MM_N],
                    wsrc[0:K, 0:128],
                    wsrc[0:K, 128:128 + MM_N],
                    start=True, stop=True,
                )

            raccs = racc_pool.tile([128, N_BJ * GROUP_COLS], mybir.dt.float16,
                                   tag="raccs")
            m1_sb = out_pool.tile([128, N_BJ], mybir.dt.float32, tag="m1")

            gp_ctr = [0]
            for g in range(N_GROUPS):
                acc_g = acc_pool.tile([128, GROUP_COLS], mybir.dt.float16,
                                      tag="acc")
                for bj in range(N_BJ):
                    pt = psum_pool.tile([128, GROUP_COLS], mybir.dt.float32,
                                        tag="ps")
                    for c in range(GROUP_COLS // MM_N):
                        col0 = g * GROUP_COLS + c * MM_N
                        nc.tensor.matmul(
                            pt[:, c * MM_N:(c + 1) * MM_N],
                            p2w_sb[:, bj * 128:(bj + 1) * 128],
                            p1m_sb[:, col0:col0 + MM_N],
                            start=True, stop=True,
                        )
                    f16 = f16_pool.tile([128, GROUP_COLS], mybir.dt.float16,
                                        tag="f16")
                    nc.scalar.copy(f16[:], pt[:])

                    # Col path: acc_g <- min(acc_g, tile) over the 16 bj.
                    if bj == 0:
                        nc.vector.tensor_copy(acc_g[:], f16[:])
                    else:
                        gp_ctr[0] += 1
                        eng = (nc.gpsimd if GP_EVERY and
                               gp_ctr[0] % GP_EVERY == 0 else nc.vector)
                        eng.tensor_tensor(
                            out=acc_g[:], in0=acc_g[:], in1=f16[:],
                            op=mybir.AluOpType.min,
                        )

                    # Row path: racc_bj <- min(racc_bj, tile) over the 8
                    # groups; the last group emits the reduced row-min.
                    rb = raccs[:, bj * GROUP_COLS:(bj + 1) * GROUP_COLS]
                    if g == 0:
                        nc.vector.tensor_copy(rb, f16[:])
                    else:
                        nc.vector.tensor_tensor(
                            out=rb, in0=rb, in1=f16[:],
                            op=mybir.AluOpType.min,
                        )
                    if g == N_GROUPS - 1:
                        # tensor_tensor_reduce doesn't compile on this
                        # walrus; fold rb in place (2x fp16) + short reduce.
                        for half in (1024, 512, 256):
                            nc.vector.tensor_tensor(
                                out=rb[:, :half], in0=rb[:, :half],
                                in1=rb[:, half:2 * half],
                                op=mybir.AluOpType.min,
                            )
                        nc.vector.tensor_reduce(
                            out=m1_sb[:, bj:bj + 1], in_=rb[:, :256],
                            axis=mybir.AxisListType.X, op=mybir.AluOpType.min,
                        )

                nc.sync.dma_start(accs[:, g * GROUP_COLS:(g + 1) * GROUP_COLS],
                                  acc_g[:])

            nc.sync.dma_start(m1[:, :], m1_sb[:])

    _legalize_waits(nc)
    return nc


def _legalize_waits(nc):
    """Walrus's per-instruction structs carry at most one sem-wait, but
    Tile's sem assignment can emit several (slot-recycle WAR + input RAW).

    1. Same-engine waits are dropped when a cross-engine wait remains:
       engines execute in order and the cross-engine consumer they wait
       on transitively waited on those same-engine ticks.
    2. Any instruction still carrying N>1 waits gets N-1 same-engine
       NoOps injected right before it, one overflow wait each.
    """
    blocks = nc.m.functions[0].blocks

    # 1. same-engine strip
    for blk in blocks:
        for ins in blk.instructions:
            si = ins.sync_info
            if si is None or len(si.on_wait) <= 1 or not si.on_update:
                continue
            self_eng = si.on_update[0].ant_name.split("_")[0]
            keep = [w for w in si.on_wait
                    if w.ant_name.split("_")[0] != self_eng]
            if keep and len(keep) < len(si.on_wait):
                si.on_wait = keep
                ins.sync_info = si

    # 2. split remaining multi-waits onto same-engine NoOps
    eng_by_prefix = {
        "PE": mybir.EngineType.PE,
        "DVE": mybir.EngineType.DVE,
        "ACT": mybir.EngineType.Activation,
        "POOL": mybir.EngineType.Pool,
        "SP": mybir.EngineType.SP,
    }
    nop_id = [0]
    for blk in blocks:
        new_list = []
        changed = False
        for ins in blk.instructions:
            si = ins.sync_info
            if si is not None and len(si.on_wait) > 1:
                eng = getattr(ins, "engine", None)
                if eng is None and si.on_update:
                    eng = eng_by_prefix.get(
                        si.on_update[0].ant_name.split("_")[0])
                if eng is None:
                    eng = mybir.EngineType.SP
                waits = list(si.on_wait)
                for w in waits[:-1]:
                    nop_id[0] += 1
                    nop = mybir.InstNoOp(
                        name=f"I-waitnop-{nop_id[0]}", ins=[], outs=[],
                        engine=eng,
                        sync_info=mybir.SyncInfo(on_wait=[w], on_update=[]),
                    )
                    new_list.append(nop)
                si.on_wait = [waits[-1]]
                ins.sync_info = si
                changed = True
            new_list.append(ins)
        if changed:
            blk.instructions = new_list


def _split3(x):
    """fp32 -> three bf16 terms with x ~= h + m + l (residual ~2^-24 |x|)."""
    h = x.astype(BF16)
    r = x - h.astype(np.float32)
    m = r.astype(BF16)
    l = (r - m.astype(np.float32)).astype(BF16)
    return h, m, l


def _weight_rows(p):
    """pc2 side (stationary): [24, N] bf16 rows carrying -2*SCALE products
    and the SCALE*|p|^2 / SCALE*ones terms of the augmented contraction."""
    x, y, z = p[:, 0], p[:, 1], p[:, 2]
    sq = (x * x + y * y + z * z).astype(np.float32)
    rows = []
    for c in (x, y, z):
        h, m, l = _split3(c)
        # pairs with moving rows (h,h,m,h,m,l): (h,h)(m,h)(h,m)(l,h)(m,m)(h,l)
        rows += [-2 * SCALE * h, -2 * SCALE * m, -2 * SCALE * h,
                 -2 * SCALE * l, -2 * SCALE * m, -2 * SCALE * h]
    ones = np.ones_like(sq)
    rows += [SCALE * ones] * 3 + list(_split3(SCALE * sq))
    return np.stack(rows).astype(BF16)


def _moving_rows(p):
    """pc1 side (moving): [24, N] bf16 rows pairing with _weight_rows."""
    x, y, z = p[:, 0], p[:, 1], p[:, 2]
    sq = (x * x + y * y + z * z).astype(np.float32)
    rows = []
    for c in (x, y, z):
        h, m, l = _split3(c)
        rows += [h, h, m, h, m, l]
    ones = np.ones_like(sq)
    rows += list(_split3(sq)) + [ones] * 3
    return np.stack(rows).astype(BF16)


def kernel(pc1, pc2):
    global _NC_CACHE, LAST_RESULTS
    p1 = np.asarray(pc1, dtype=np.float32).reshape(-1, 3)
    p2 = np.asarray(pc2, dtype=np.float32).reshape(-1, 3)
    assert p1.shape == (N1, 3) and p2.shape == (N_CORES * N_SHARD, 3)

    p1m_np = _moving_rows(p1)  # [24, 16384], shared by all cores

    in_maps = []
    for c in range(N_CORES):
        shard = p2[c * N_SHARD:(c + 1) * N_SHARD]
        packed = np.concatenate([_weight_rows(shard), p1m_np], axis=1)
        in_maps.append({"inp": np.ascontiguousarray(packed)})

    if _NC_CACHE is None:
        _NC_CACHE = _build_nc()

    res = run_bass_kernel_spmd(
        _NC_CACHE, in_maps, core_ids=list(range(N_CORES)), trace=TRACE,
    )
    LAST_RESULTS = res

    # dist1 (per pc2 point over all pc1): m1[p, bj] for pc2 idx
    # c*2048 + bj*128 + p -- complete on device.
    d2_1 = np.concatenate(
        [r["m1"].T.reshape(-1) for r in res.results])  # [16384] pc2-major
    # dist2 (per pc1 point over all pc2): host min over cores x partitions.
    acc = np.stack([r["accs"] for r in res.results])  # [8, 128, 16384] fp16
    d2_2 = acc.reshape(N_CORES * 128, N1).min(axis=0).astype(np.float32)

    dist1 = np.sqrt(np.maximum(d2_1 / SCALE, 0.0))
    dist2 = np.sqrt(np.maximum(d2_2 / SCALE, 0.0))
    return np.asarray(dist1.mean() + dist2.mean(), dtype=np.float32)
